# revision 1
# baseline (speedup 1.0000x reference)
"""BiLSTM encoder Trainium2 kernel (8-core SPMD, batch-data-parallel).

Strategy
--------
- Shard batch B=128 across 8 cores (16 per core); replicate weights.
- Host folds the input projection into the gate projection:
      pre = x @ (Wih @ W_proj)^T + (Wih @ b_proj + b)
- Device per core:
    Phase A (GEMM): pre[dir] = x_shard @ Wc[dir]^T + bc (bf16 in SBUF,
      fp32 psum), x transposed on the fly via bf16 cast + xbar DMA.
    Phase B (scan): layout-C recurrence. Gates [128 feat, 4x16] in PSUM:
      identity-matmul accumulates pre, 4 small matmuls add Whh @ h.
      All gate nonlinearities via ONE tanh (i,f,o rows pre-scaled by 0.5
      so sigmoid(x) = 0.5*tanh(x/2)+0.5 is an affine fixup fused into
      the vector ops via affine_mul_reduce).
    Phase C (LayerNorm): hs transposed back to row space via xbar DMA,
      unit-LN per row; gamma/beta applied on host only if not identity.
"""

import sys

for _p in ("/opt/trn_rl_repo", "/opt/pypackages"):
    if _p not in sys.path:
        sys.path.insert(0, _p)

from contextlib import ExitStack

import ml_dtypes
import numpy as np

import concourse.bacc as bacc
import concourse.mybir as mybir
import concourse.tile as tile
from concourse.bass_utils import run_bass_kernel_spmd

BF = mybir.dt.bfloat16
F32 = mybir.dt.float32
AF = mybir.ActivationFunctionType
ALU = mybir.AluOpType

H = 128
DD = 256
G4 = 512
N_CORES = 8

_BUILD_CACHE = {}


def build_nc(L=512, BL=16, n_cores=N_CORES):
    key = (L, BL, n_cores)
    if key in _BUILD_CACHE:
        return _BUILD_CACHE[key]
    R = BL * L  # rows per core (row index = t*BL + b)
    CH = 512 if R >= 512 else R  # GEMM chunk rows
    NCH = R // CH
    RT = CH // 128  # 128-row tiles per chunk
    nc = bacc.Bacc("TRN2", target_bir_lowering=False, debug=False, num_devices=n_cores)

    xs = nc.dram_tensor("xs", [BL, L, DD], F32, kind="ExternalInput").ap()
    wc_d = {
        d: [
            nc.dram_tensor(f"wc_{d}{k}", [128, G4], BF, kind="ExternalInput").ap()
            for k in range(2)
        ]
        for d in "fb"
    }
    whh_d = {
        d: nc.dram_tensor(f"whh_{d}", [128, G4], BF, kind="ExternalInput").ap()
        for d in "fb"
    }
    bc_d = {
        d: nc.dram_tensor(f"bc_{d}", [128, 4], F32, kind="ExternalInput").ap()
        for d in "fb"
    }
    ident_d = nc.dram_tensor("ident", [128, 128], BF, kind="ExternalInput").ap()
    y = nc.dram_tensor("y", [BL, L, 2 * H], F32, kind="ExternalOutput").ap()

    x3 = xs.rearrange("b l d -> l b d")  # row-major (t, b) when flattened
    y3 = y.rearrange("b l f -> l b f")

    with tile.TileContext(nc) as tc:
        with ExitStack() as ctx:
            # --- persistent pools (whole kernel) ---
            wpool = ctx.enter_context(tc.tile_pool(name="w", bufs=1))
            prepool = ctx.enter_context(tc.tile_pool(name="pre", bufs=1))
            hspool = ctx.enter_context(tc.tile_pool(name="hs", bufs=1))
            stpool = ctx.enter_context(tc.tile_pool(name="state", bufs=1))

            # load weights to SBUF
            wc_sb = {}
            whh_sb = {}
            bc_sb = {}
            for d in "fb":
                wc_sb[d] = []
                for k in range(2):
                    t_ = wpool.tile([128, G4], BF, tag=f"wc{d}{k}")
                    nc.sync.dma_start(t_[:], wc_d[d][k])
                    wc_sb[d].append(t_)
                t_ = wpool.tile([128, G4], BF, tag=f"whh{d}")
                nc.sync.dma_start(t_[:], whh_d[d])
                whh_sb[d] = t_
                t_ = wpool.tile([128, 4], F32, tag=f"bc{d}")
                nc.sync.dma_start(t_[:], bc_d[d])
                bc_sb[d] = t_
            ident = wpool.tile([128, 128], BF, tag="ident")
            nc.sync.dma_start(ident[:], ident_d)

            # big SBUF tensors
            pre = {d: prepool.tile([128, L * 64], BF, tag=f"pre{d}", name=f"pre{d}") for d in "fb"}
            hs = {d: hspool.tile([128, R], BF, tag=f"hs{d}", name=f"hs{d}") for d in "fb"}
            c_sb = {d: stpool.tile([128, BL], F32, tag=f"c{d}", name=f"c{d}") for d in "fb"}

            # 4D views of pre: [p, t, m, b]
            pre_v = {
                d: pre[d][:].rearrange("p (t m b) -> p t m b", m=4, b=BL) for d in "fb"
            }

            # ------- Fused Phase A (pre GEMM) + Phase B (scan) -------
            # GEMM chunks are produced ping-pong from both ends (0, NCH-1,
            # 1, NCH-2, ...) so the forward scan gets pre chunk 0 and the
            # backward scan gets pre chunk NCH-1 early; the scan then
            # overlaps the remaining GEMM work.
            xf32p = ctx.enter_context(tc.tile_pool(name="xf32", bufs=6))
            xbfp = ctx.enter_context(tc.tile_pool(name="xbf", bufs=6))
            xtp = ctx.enter_context(tc.tile_pool(name="xt", bufs=4))
            gpsum = ctx.enter_context(tc.tile_pool(name="gpsum", bufs=2, space="PSUM"))
            spsum = {
                d: ctx.enter_context(tc.tile_pool(name=f"ps{d}", bufs=3, space="PSUM"))
                for d in "fb"
            }
            tap = ctx.enter_context(tc.tile_pool(name="ta", bufs=4))
            uvp = ctx.enter_context(tc.tile_pool(name="uv", bufs=4))
            thp = ctx.enter_context(tc.tile_pool(name="th", bufs=4))
            junkp = ctx.enter_context(tc.tile_pool(name="junk", bufs=8))

            def emit_gemm_chunk(ch):
                xt = [
                    xtp.tile([128, CH], BF, tag=f"xt{k}", name=f"xt{k}")
                    for k in range(2)
                ]
                for rt in range(RT):
                    l0 = (ch * CH + rt * 128) // BL
                    nl = 128 // BL
                    xa = xf32p.tile([128, DD], F32, name="xa")
                    nc.gpsimd.dma_start(xa[:], x3[l0 : l0 + nl])
                    xb = xbfp.tile([128, DD], BF, name="xb")
                    if rt % 2 == 0:
                        nc.scalar.activation(xb[:], xa[:], AF.Copy)
                    else:
                        nc.vector.tensor_copy(xb[:], xa[:])
                    for k in range(2):
                        eng = nc.scalar if k == 0 else nc.sync
                        eng.dma_start_transpose(
                            xt[k][:, rt * 128 : (rt + 1) * 128],
                            xb[:, k * 128 : (k + 1) * 128],
                        )
                t0 = ch * (CH // BL)
                nt = CH // BL
                for d in "fb":
                    for m in range(4):
                        ps = gpsum.tile([128, CH], F32, name="gps")
                        nc.tensor.matmul(
                            ps[:],
                            wc_sb[d][0][:, m * 128 : (m + 1) * 128],
                            xt[0][:],
                            start=True,
                            stop=False,
                        )
                        nc.tensor.matmul(
                            ps[:],
                            wc_sb[d][1][:, m * 128 : (m + 1) * 128],
                            xt[1][:],
                            start=False,
                            stop=True,
                        )
                        dst = pre_v[d][:, t0 : t0 + nt, m, :]
                        bias_ap = bc_sb[d][:, m : m + 1]
                        if (m + (d == "b")) % 2 == 0:
                            nc.scalar.activation(dst, ps[:], AF.Identity, bias=bias_ap)
                        else:
                            nc.vector.tensor_scalar_add(dst, ps[:], bias_ap)

            gemm_order = []
            lo, hi = 0, NCH - 1
            while lo <= hi:
                gemm_order.append(lo)
                if hi != lo:
                    gemm_order.append(hi)
                lo += 1
                hi -= 1
            for ch in gemm_order:
                emit_gemm_chunk(ch)

            # --- scan (dir b runs a half-step behind dir f in emission
            # order so ACT/DVE ping-pong between the two chains) ---
            def emit_mms(d, s):
                t = s if d == "f" else L - 1 - s
                t_prev = (s - 1) if d == "f" else (L - s)
                ps = spsum[d].tile([128, 64], F32, name=f"ps{d}")
                nc.tensor.matmul(
                    ps[:],
                    ident[:],
                    pre[d][:, t * 64 : (t + 1) * 64],
                    start=True,
                    stop=True,
                )
                if s > 0:
                    h_prev = hs[d][:, t_prev * BL : (t_prev + 1) * BL]
                    for m in range(4):
                        nc.tensor.matmul(
                            ps[:, m * BL : (m + 1) * BL],
                            whh_sb[d][:, m * 128 : (m + 1) * 128],
                            h_prev,
                            start=False,
                            stop=True,
                            skip_group_check=True,
                        )
                return ps, t

            def emit_tanh_all(d, ps):
                ta = tap.tile([128, 64], F32, tag=f"ta{d}", name=f"ta{d}")
                nc.scalar.activation(ta[:], ps[:], AF.Tanh)
                return ta

            def emit_uva(d, s, ta):
                u = uvp.tile([128, BL], F32, tag=f"u{d}", name=f"u{d}")
                ju = junkp.tile([128, 1], F32, name="ju")
                nc.vector.affine_mul_reduce(
                    u[:], ju[:], ta[:, 0:BL], ta[:, 48:64], 0.5, 0.5
                )
                if s == 0:
                    nc.vector.tensor_copy(c_sb[d][:], u[:])
                else:
                    v = uvp.tile([128, BL], F32, tag=f"v{d}", name=f"v{d}")
                    jv = junkp.tile([128, 1], F32, name="jv")
                    nc.vector.affine_mul_reduce(
                        v[:], jv[:], ta[:, BL : 2 * BL], c_sb[d][:], 0.5, 0.5
                    )
                    nc.vector.tensor_add(c_sb[d][:], u[:], v[:])

            def emit_tc(d):
                th = thp.tile([128, BL], F32, tag=f"th{d}", name=f"th{d}")
                nc.scalar.activation(th[:], c_sb[d][:], AF.Tanh)
                return th

            def emit_h(d, t, ta, th):
                jh = junkp.tile([128, 1], F32, name="jh")
                nc.vector.affine_mul_reduce(
                    hs[d][:, t * BL : (t + 1) * BL],
                    jh[:],
                    ta[:, 2 * BL : 3 * BL],
                    th[:],
                    0.5,
                    0.5,
                )

            # --- LayerNorm emission (overlapped into the scan tail) ---
            xrp = ctx.enter_context(tc.tile_pool(name="xr", bufs=8))
            scp = ctx.enter_context(tc.tile_pool(name="scr", bufs=2))
            statp = ctx.enter_context(tc.tile_pool(name="stat", bufs=6))
            outp = ctx.enter_context(tc.tile_pool(name="out", bufs=4))
            NF = 2 * H
            eps = statp.tile([128, 1], F32, tag="eps", bufs=1, name="eps")
            nc.vector.memset(eps[:], 1e-5)

            def emit_ln_xbar(cc):
                xr = xrp.tile([128, NF], BF, name="xr")
                for di, d in enumerate("fb"):
                    eng = nc.scalar if di == 0 else nc.sync
                    eng.dma_start_transpose(
                        xr[:, di * H : (di + 1) * H],
                        hs[d][:, cc * 128 : (cc + 1) * 128],
                    )
                return xr

            def emit_ln_chunk(cc, xr):
                s1 = statp.tile([128, 1], F32, tag="s1", name="s1")
                nc.vector.tensor_reduce(
                    s1[:], xr[:], axis=mybir.AxisListType.X, op=ALU.add
                )
                scr = scp.tile([128, NF], F32, name="scr")
                s2 = statp.tile([128, 1], F32, tag="s2", name="s2")
                nc.scalar.activation(scr[:], xr[:], AF.Square, accum_out=s2[:])
                mu = statp.tile([128, 1], F32, tag="mu", name="mu")
                nc.vector.tensor_scalar_mul(mu[:], s1[:], 1.0 / NF)
                e2 = statp.tile([128, 1], F32, tag="e2", name="e2")
                nc.vector.tensor_scalar_mul(e2[:], s2[:], 1.0 / NF)
                mu2 = statp.tile([128, 1], F32, tag="mu2", name="mu2")
                nc.vector.tensor_mul(mu2[:], mu[:], mu[:])
                var = statp.tile([128, 1], F32, tag="var", name="var")
                nc.vector.scalar_tensor_tensor(
                    var[:], mu2[:], -1.0, e2[:], ALU.mult, ALU.add
                )
                sd = statp.tile([128, 1], F32, tag="sd", name="sd")
                nc.scalar.activation(sd[:], var[:], AF.Sqrt, bias=eps[:])
                r = statp.tile([128, 1], F32, tag="r", name="r")
                nc.vector.reciprocal(r[:], sd[:])
                nmr = statp.tile([128, 1], F32, tag="nmr", name="nmr")
                nc.vector.scalar_tensor_tensor(
                    nmr[:], mu[:], -1.0, r[:], ALU.mult, ALU.mult
                )
                ot = outp.tile([128, NF], F32, name="ot")
                nc.vector.tensor_scalar(ot[:], xr[:], r[:], nmr[:], ALU.mult, ALU.add)
                nl = 128 // BL
                nc.sync.dma_start(y3[cc * nl : (cc + 1) * nl], ot[:])

            # readiness: chunk cc needs hs_f t<8(cc+1) (s >= 8cc+7) and
            # hs_b t>=8cc (s >= 511-8cc)
            ln_at = {}
            ln_post = []
            for cc in range(R // 128):
                # hs_f block done at s=8cc+7; hs_b block's last write (t=8cc,
                # s=L-1-8cc) is EMITTED one iteration later due to the skew.
                s_ready = max(8 * cc + 7, L - 8 * cc)
                if s_ready <= L - 1 - 4:
                    ln_at.setdefault(s_ready, []).append(cc)
                else:
                    ln_post.append(cc)

            pend = None  # (ta_b, t_b) awaiting tail emission
            ln_fly = []
            for s in range(L):
                ps_f, t_f = emit_mms("f", s)
                ta_f = emit_tanh_all("f", ps_f)
                if pend is not None:
                    ta_bp, t_bp = pend
                    th_b = emit_tc("b")
                    emit_h("b", t_bp, ta_bp, th_b)
                ps_b, t_b = emit_mms("b", s)
                emit_uva("f", s, ta_f)
                ta_b = emit_tanh_all("b", ps_b)
                emit_uva("b", s, ta_b)
                th_f = emit_tc("f")
                emit_h("f", t_f, ta_f, th_f)
                pend = (ta_b, t_b)
                for cc in ln_at.get(s, []):
                    ln_fly.append((cc, emit_ln_xbar(cc)))
                if s - 4 in ln_at:
                    for cc in ln_at[s - 4]:
                        cc2, xr2 = ln_fly.pop(0)
                        assert cc2 == cc
                        emit_ln_chunk(cc, xr2)
            ta_bp, t_bp = pend
            th_b = emit_tc("b")
            emit_h("b", t_bp, ta_bp, th_b)
            for cc, xr2 in ln_fly:
                emit_ln_chunk(cc, xr2)
            for cc in ln_post:
                emit_ln_chunk(cc, emit_ln_xbar(cc))

    nc.compile()
    _BUILD_CACHE[key] = nc
    return nc


def _prep_weights(W_proj, b_proj, Wih, Whh, b):
    """Host-side: fold projection, permute gates to (i,f,o,g), pre-scale
    i/f/o rows by 0.5 (sigmoid-via-tanh trick), build lhsT layouts."""
    perm = np.r_[0:256, 384:512, 256:384]
    scale = np.concatenate([np.full(384, 0.5), np.ones(128)]).astype(np.float64)
    Wc = (Wih.astype(np.float64) @ W_proj.astype(np.float64))[perm] * scale[:, None]
    bc = (Wih.astype(np.float64) @ b_proj.astype(np.float64) + b.astype(np.float64))[
        perm
    ] * scale
    Whh_p = Whh[perm].astype(np.float64) * scale[:, None]
    bf16 = ml_dtypes.bfloat16
    WcT = np.ascontiguousarray(Wc.T.astype(np.float32).astype(bf16))  # [D, 4H]
    WhhT = np.ascontiguousarray(Whh_p.T.astype(np.float32).astype(bf16))  # [H, 4H]
    bc128 = np.ascontiguousarray(bc.astype(np.float32).reshape(4, 128).T)  # [128, 4]
    return WcT, WhhT, bc128


def kernel(x, W_proj, b_proj, Wih_f, Whh_f, b_f, Wih_b, Whh_b, b_b, gamma, beta):
    x = np.asarray(x, dtype=np.float32)
    B, L, D = x.shape
    BL = B // N_CORES
    nc = build_nc(L=L, BL=BL)

    bf16 = ml_dtypes.bfloat16
    in_common = {"ident": np.eye(128, dtype=np.float32).astype(bf16)}
    for d, Wih, Whh, b in (("f", Wih_f, Whh_f, b_f), ("b", Wih_b, Whh_b, b_b)):
        WcT, WhhT, bc128 = _prep_weights(
            np.asarray(W_proj), np.asarray(b_proj), np.asarray(Wih), np.asarray(Whh),
            np.asarray(b),
        )
        in_common[f"wc_{d}0"] = np.ascontiguousarray(WcT[0:128])
        in_common[f"wc_{d}1"] = np.ascontiguousarray(WcT[128:256])
        in_common[f"whh_{d}"] = WhhT
        in_common[f"bc_{d}"] = bc128

    in_maps = [
        {**in_common, "xs": np.ascontiguousarray(x[i * BL : (i + 1) * BL])}
        for i in range(N_CORES)
    ]
    res = run_bass_kernel_spmd(nc, in_maps, list(range(N_CORES)))
    out = np.concatenate([res.results[i]["y"] for i in range(N_CORES)], axis=0)

    gamma = np.asarray(gamma, dtype=np.float32)
    beta = np.asarray(beta, dtype=np.float32)
    if not (np.all(gamma == 1.0) and np.all(beta == 0.0)):
        out = out * gamma + beta
    return out.astype(np.float32)


if __name__ == "__main__":
    d = np.load("/root/problem/ref.npz")
    inp = {k: d[k] for k in d.files if k != "exp"}
    got = kernel(**inp)
    exp = d["exp"]
    rel = np.linalg.norm(got - exp) / np.linalg.norm(exp)
    print("rel fro:", rel, "maxabs:", np.abs(got - exp).max())

